# revision 9
# baseline (speedup 1.0000x reference)
"""Attention pooling kernel for Trainium2 (8 NeuronCores), v6.

Computes: scores = E @ q; w = softmax(scores); out = w @ E
for E [N=2097152, 64] fp32, q [64] fp32.

v9 = v8 + prod bufs=4 + psumf bufs=4:
  - 8-row packing: partition k = (r, dh), r in [0,8), dh in [0,16);
    d split into 4 quarters j; scores accumulate 4 matmuls (qmat_j)
    into PSUM with start/stop flags.
  - DRAM layout [k, (t, e, j, c)]: DMA tile t (DT=4096 n-cols) is
    fetched as 4 strips of 1MB (one per exp-tile e of ET=1024), so the
    first matmuls start after ~1MB instead of 4MB, and 32 smaller
    transfers pack the 16 DMA engines more smoothly.
  - ep tiles are 4D [128, e, j, c]; the weighted-sum unit for quarter
    j reads the strided view ep[:, :, j, :] (innermost contiguous, so
    DVE 2x mode is preserved).
  - Weighted-sum routes (A=DVE stt fused, H=DVE 2x mult + PE 16:1
    identity-matmul PSUM fold + small DVE reduce, C=DVE 2x mult + ACT
    Copy accum_out reduce), mix A2/H18/C12, all fp16.
  - Host combine identical to v3.
"""

import sys

sys.path.insert(0, "/opt/trn_rl_repo")

import numpy as np

N_TOTAL = 2097152
D = 64
N_CORES = 8
N_PER_CORE = N_TOTAL // N_CORES          # 262144
R = 8                                    # rows packed per n-column
NQ = 4                                   # d quarters
DQ = D // NQ                             # 16 d's per quarter
NCOLS = N_PER_CORE // R                  # 32768 n-columns per core
MM_N = 512                               # scores matmul free dim
DT = 4096                                # n-columns per DMA tile
ET = 1024                                # n-columns per exp/PSUM tile
NE = DT // ET                            # 4 strips per tile
N_TILES = NCOLS // DT                    # 8
FOLD = 8                                 # H-route fold ratio

# route per unit (unit = t * NQ + j):
# A=DVE stt, H=DVE mult + PE fold + DVE reduce, C=DVE mult + ACT reduce
ROUTES = list("HCHAHCHC" * 4)
ROUTES[3] = "H"
ROUTES[19] = "H"                         # -> A2 H18 C12

_compiled = {}


def _build_nc(ncols, dt, et, routes):
    import concourse.bacc as bacc
    import concourse.bass as bass
    import concourse.mybir as mybir
    import concourse.tile as tile

    fp32 = mybir.dt.float32
    fp16 = mybir.dt.float16

    n_tiles = ncols // dt
    ne = dt // et
    blk = NQ * dt                        # elements per DMA tile per partition
    strip = NQ * et                      # elements per strip per partition

    nc = bacc.Bacc()
    ep_dram = nc.declare_dram_parameter("epack", [128, NQ * ncols], fp16, isOutput=False)
    qmat_dram = nc.declare_dram_parameter("qmat", [128, NQ * 128], fp16, isOutput=False)
    idmat_dram = nc.declare_dram_parameter("idmat", [128, 128], fp16, isOutput=False)
    cshift_dram = nc.declare_dram_parameter("cshift", [128, 1], fp32, isOutput=False)
    out_dram = nc.declare_dram_parameter("out", [128, NQ + 1], fp32, isOutput=True)

    with tile.TileContext(nc) as tc:
        with (
            tc.tile_pool(name="const", bufs=1) as const_pool,
            tc.tile_pool(name="ep", bufs=4) as ep_pool,
            tc.tile_pool(name="w", bufs=3) as w_pool,
            tc.tile_pool(name="junk", bufs=1) as junk_pool,
            tc.tile_pool(name="prod", bufs=4) as prod_pool,
            tc.tile_pool(name="acc", bufs=1) as acc_pool,
            tc.tile_pool(name="psum", bufs=2, space=bass.MemorySpace.PSUM) as psum_pool,
            tc.tile_pool(name="psumf", bufs=4, space=bass.MemorySpace.PSUM) as psumf_pool,
        ):
            qmat = const_pool.tile([128, NQ * 128], fp16, tag="qmat")
            idmat = const_pool.tile([128, 128], fp16, tag="idmat")
            cshift = const_pool.tile([128, 1], fp32, tag="cshift")
            nc.sync.dma_start(qmat[:], qmat_dram[:])
            nc.sync.dma_start(idmat[:], idmat_dram[:])
            nc.sync.dma_start(cshift[:], cshift_dram[:])

            n_acts = ncols // et
            master_se = acc_pool.tile([128, n_acts], fp32, tag="master_se")
            master_aw = acc_pool.tile([128, NQ * n_tiles], fp32, tag="master_aw")

            for t in range(n_tiles):
                ep = ep_pool.tile([128, ne, NQ, et], fp16, tag="ep")
                for e in range(ne):
                    nc.sync.dma_start(
                        ep[:, e, :, :].rearrange("p j c -> p (j c)"),
                        ep_dram[:, t * blk + e * strip:t * blk + (e + 1) * strip],
                    )

                w_sb = w_pool.tile([128, ne, et], fp16, tag="w")
                for e in range(ne):
                    ps = psum_pool.tile([128, et], fp32, tag="ps")
                    for j in range(NQ):
                        for c0 in range(0, et, MM_N):
                            nc.tensor.matmul(
                                ps[:, c0:c0 + MM_N],
                                qmat[:, j * 128:(j + 1) * 128],
                                ep[:, e, j, c0:c0 + MM_N],
                                start=(j == 0),
                                stop=(j == NQ - 1),
                            )
                    nc.scalar.activation(
                        w_sb[:, e, :],
                        ps[:],
                        mybir.ActivationFunctionType.Exp,
                        bias=cshift[:, 0:1],
                        scale=1.0,
                        accum_out=master_se[:, t * ne + e:t * ne + e + 1],
                    )

                for j in range(NQ):
                    unit = t * NQ + j
                    route = routes[unit]
                    slot = master_aw[:, j * n_tiles + t:j * n_tiles + t + 1]
                    epj = ep[:, :, j, :]                    # [128, ne, et] strided
                    wv = w_sb[:, :, :]                      # [128, ne, et] contig
                    if route == "A":
                        junk = junk_pool.tile([128, ne, et], fp16, tag="junk")
                        nc.vector.scalar_tensor_tensor(
                            junk[:], epj, 1.0, wv,
                            op0=mybir.AluOpType.mult,
                            op1=mybir.AluOpType.mult,
                            accum_out=slot,
                        )
                    else:
                        prod = prod_pool.tile([128, dt], fp16, tag="prod")
                        nc.vector.tensor_tensor(
                            prod[:].rearrange("p (e c) -> p e c", e=ne, c=et),
                            epj, wv, op=mybir.AluOpType.mult)
                        if route == "H":
                            fw = dt // FOLD
                            psf = psumf_pool.tile([128, fw], fp32, tag="psf")
                            for g in range(FOLD):
                                nc.tensor.matmul(
                                    psf[:],
                                    idmat[:],
                                    prod[:, g * fw:(g + 1) * fw],
                                    start=(g == 0),
                                    stop=(g == FOLD - 1),
                                )
                            nc.vector.tensor_reduce(
                                slot, psf[:], axis=mybir.AxisListType.X,
                                op=mybir.AluOpType.add,
                            )
                        else:  # C: ACT copy-accumulate reduce
                            junk2 = junk_pool.tile([128, dt], fp16, tag="junk2")
                            nc.scalar.activation(
                                junk2[:], prod[:],
                                mybir.ActivationFunctionType.Copy,
                                scale=1.0,
                                accum_out=slot,
                            )

            res = acc_pool.tile([128, NQ + 1], fp32, tag="res")
            for j in range(NQ):
                nc.vector.tensor_reduce(
                    res[:, j:j + 1],
                    master_aw[:, j * n_tiles:(j + 1) * n_tiles],
                    axis=mybir.AxisListType.X,
                    op=mybir.AluOpType.add,
                )
            nc.vector.tensor_reduce(
                res[:, NQ:NQ + 1], master_se[:], axis=mybir.AxisListType.X,
                op=mybir.AluOpType.add,
            )
            nc.sync.dma_start(out_dram[:], res[:])

    nc.compile()
    return nc


def _pack_core(e_core, dt, et):
    # [Nc, 64] -> A[r, dh, j, n] = E[8n+r, 16j+dh] -> [k, (t, e, j, c)]
    ncols = e_core.shape[0] // R
    n_tiles = ncols // dt
    ne = dt // et
    a = e_core.reshape(ncols, R, NQ, DQ).transpose(1, 3, 2, 0)     # [r, dh, j, n]
    a = a.reshape(128, NQ, n_tiles, ne, et).transpose(0, 2, 3, 1, 4)  # [k,t,e,j,c]
    return np.ascontiguousarray(a.reshape(128, NQ * ncols)).astype(np.float16)


def _make_consts(query):
    c_shift = float(6.0 * np.linalg.norm(query.astype(np.float64)))
    q16 = query.astype(np.float16)
    qmat = np.zeros((128, NQ * 128), dtype=np.float16)
    for j in range(NQ):
        for r in range(R):
            qmat[r * DQ:(r + 1) * DQ, j * 128 + r * DQ:j * 128 + (r + 1) * DQ] = \
                q16[j * DQ:(j + 1) * DQ][:, None]
    cshift = np.full((128, 1), -c_shift, dtype=np.float32)
    idmat = np.eye(128, dtype=np.float16)
    return qmat, cshift, idmat


def get_nc():
    key = (NCOLS, DT, ET, "".join(ROUTES))
    if key not in _compiled:
        _compiled[key] = _build_nc(NCOLS, DT, ET, ROUTES)
    return _compiled[key]


def make_in_maps(embeddings, query):
    embeddings = np.asarray(embeddings, dtype=np.float32)
    query = np.asarray(query, dtype=np.float32)
    qmat, cshift, idmat = _make_consts(query)
    in_maps = []
    for c in range(N_CORES):
        e_core = embeddings[c * N_PER_CORE:(c + 1) * N_PER_CORE]
        in_maps.append({
            "epack": _pack_core(e_core, DT, ET),
            "qmat": qmat,
            "cshift": cshift,
            "idmat": idmat,
        })
    return in_maps


def combine(results):
    wsum = np.zeros(D, dtype=np.float64)
    sumexp = 0.0
    for r_ in results:
        out = r_["out"].astype(np.float64)          # [128, 5]
        for j in range(NQ):
            wsum[j * DQ:(j + 1) * DQ] += out[:, j].reshape(R, DQ).sum(axis=0)
        sumexp += out[::DQ, NQ].sum()               # one rep per r-block
    return (wsum / sumexp).astype(np.float32)


def kernel(embeddings, query):
    from concourse.bass_utils import run_bass_kernel_spmd

    nc = get_nc()
    in_maps = make_in_maps(embeddings, query)

    res = None
    for attempt in range(3):
        try:
            res = run_bass_kernel_spmd(nc, in_maps, list(range(N_CORES)))
            break
        except Exception:
            if attempt == 2:
                raise
    return combine(res.results)


# revision 11
# speedup vs baseline: 1.0231x; 1.0231x over previous
"""Attention pooling kernel for Trainium2 (8 NeuronCores).

Computes: scores = E @ q; w = softmax(scores); out = w @ E
for E [N=2097152, 64] fp32, q [64] fp32.
Sharding: E rows split evenly across the 8 cores; each core produces
per-partition partial weighted sums + sumexp; the host sums them and
divides (flash-attention-style combine with a constant shift C = 6|q|
that cancels in the division).

Per-core design (N/8 = 262144 rows, fp16 wire format -> HBM traffic
is 2 B/elem, DMA floor ~87us/core; measured rel err ~6.5e-4 vs the
2e-2 gate):
  - 8-row packing: partition k = (r, dh), r in [0,8), dh in [0,16);
    the d axis is split into 4 quarters j.  Scores for a PSUM chunk
    accumulate 4 matmuls (stationary qmat_j per quarter) via
    start/stop flags, so the exp free-size per core is N/8 (ACT is
    off the critical path) while PE moving work stays N*D/128 cols.
  - DRAM layout [k, (t, e, j, c)]: each DMA tile t (DT=4096 n-cols,
    4 MB) is fetched as 4 strips of 1 MB (one per exp tile e of
    ET=1024), so the first matmuls start after ~1MB and 32 transfers
    pack the 16 DMA engines smoothly (DMA runs at its ~360 B/ns
    floor).
  - ep tiles are 4D [128, e, j, c]; the weighted-sum unit for quarter
    j reads the strided view ep[:, :, j, :] (innermost contiguous, so
    DVE 2x mode is preserved).
  - The 32 weighted-sum units (one per (tile, quarter), 4096 cols)
    are spread across engines by a route schedule to balance
    DVE/ACT/PE against the DMA floor:
      A: DVE scalar_tensor_tensor (fused mult+accum, 1x rate)
      H: DVE tensor_tensor mult (2x rate) -> PE 8:1 identity-matmul
         PSUM-accumulation fold -> small DVE reduce
      C: DVE tensor_tensor mult (2x) -> ACT Copy-activation accum_out
    Mix A2/H18/C12, all operands fp16 (mixed fp16/bf16 disables the
    DVE 2x mode; GPSIMD offload loses to its ~1.2us/op sem overhead).
  - Host: out[d] = sum over cores/r-blocks of the per-partition
    accumulators, divided by the summed sumexp.
"""

import sys

sys.path.insert(0, "/opt/trn_rl_repo")

import numpy as np

N_TOTAL = 2097152
D = 64
N_CORES = 8
N_PER_CORE = N_TOTAL // N_CORES          # 262144
R = 8                                    # rows packed per n-column
NQ = 4                                   # d quarters
DQ = D // NQ                             # 16 d's per quarter
NCOLS = N_PER_CORE // R                  # 32768 n-columns per core
MM_N = 512                               # scores matmul free dim
DT = 4096                                # n-columns per DMA tile
ET = 1024                                # n-columns per exp/PSUM tile
NE = DT // ET                            # 4 strips per tile
N_TILES = NCOLS // DT                    # 8
FOLD = 8                                 # H-route fold ratio

# route per unit (unit = t * NQ + j):
# A=DVE stt, H=DVE mult + PE fold + DVE reduce, C=DVE mult + ACT reduce
ROUTES = list("HCHAHCHC" * 4)
ROUTES[3] = "H"
ROUTES[19] = "H"                         # -> A2 H18 C12

_compiled = {}


def _build_nc(ncols, dt, et, routes):
    import concourse.bacc as bacc
    import concourse.bass as bass
    import concourse.mybir as mybir
    import concourse.tile as tile

    fp32 = mybir.dt.float32
    fp16 = mybir.dt.float16

    n_tiles = ncols // dt
    ne = dt // et
    blk = NQ * dt                        # elements per DMA tile per partition
    strip = NQ * et                      # elements per strip per partition

    nc = bacc.Bacc()
    ep_dram = nc.declare_dram_parameter("epack", [128, NQ * ncols], fp16, isOutput=False)
    qmat_dram = nc.declare_dram_parameter("qmat", [128, NQ * 128], fp16, isOutput=False)
    idmat_dram = nc.declare_dram_parameter("idmat", [128, 128], fp16, isOutput=False)
    cshift_dram = nc.declare_dram_parameter("cshift", [128, 1], fp32, isOutput=False)
    out_dram = nc.declare_dram_parameter("out", [128, NQ + 1], fp32, isOutput=True)

    with tile.TileContext(nc) as tc:
        with (
            tc.tile_pool(name="const", bufs=1) as const_pool,
            tc.tile_pool(name="ep", bufs=4) as ep_pool,
            tc.tile_pool(name="w", bufs=3) as w_pool,
            tc.tile_pool(name="junk", bufs=1) as junk_pool,
            tc.tile_pool(name="prod", bufs=3) as prod_pool,
            tc.tile_pool(name="acc", bufs=1) as acc_pool,
            tc.tile_pool(name="psum", bufs=2, space=bass.MemorySpace.PSUM) as psum_pool,
            tc.tile_pool(name="psumf", bufs=2, space=bass.MemorySpace.PSUM) as psumf_pool,
        ):
            qmat = const_pool.tile([128, NQ * 128], fp16, tag="qmat")
            idmat = const_pool.tile([128, 128], fp16, tag="idmat")
            cshift = const_pool.tile([128, 1], fp32, tag="cshift")
            nc.sync.dma_start(qmat[:], qmat_dram[:])
            nc.sync.dma_start(idmat[:], idmat_dram[:])
            nc.sync.dma_start(cshift[:], cshift_dram[:])

            n_acts = ncols // et
            master_se = acc_pool.tile([128, n_acts], fp32, tag="master_se")
            master_aw = acc_pool.tile([128, NQ * n_tiles], fp32, tag="master_aw")

            for t in range(n_tiles):
                ep = ep_pool.tile([128, ne, NQ, et], fp16, tag="ep")
                for e in range(ne):
                    nc.sync.dma_start(
                        ep[:, e, :, :].rearrange("p j c -> p (j c)"),
                        ep_dram[:, t * blk + e * strip:t * blk + (e + 1) * strip],
                    )

                w_sb = w_pool.tile([128, ne, et], fp16, tag="w")
                for e in range(ne):
                    ps = psum_pool.tile([128, et], fp32, tag="ps")
                    for j in range(NQ):
                        for c0 in range(0, et, MM_N):
                            nc.tensor.matmul(
                                ps[:, c0:c0 + MM_N],
                                qmat[:, j * 128:(j + 1) * 128],
                                ep[:, e, j, c0:c0 + MM_N],
                                start=(j == 0),
                                stop=(j == NQ - 1),
                            )
                    nc.scalar.activation(
                        w_sb[:, e, :],
                        ps[:],
                        mybir.ActivationFunctionType.Exp,
                        bias=cshift[:, 0:1],
                        scale=1.0,
                        accum_out=master_se[:, t * ne + e:t * ne + e + 1],
                    )

                for j in range(NQ):
                    unit = t * NQ + j
                    route = routes[unit]
                    slot = master_aw[:, j * n_tiles + t:j * n_tiles + t + 1]
                    epj = ep[:, :, j, :]                    # [128, ne, et] strided
                    wv = w_sb[:, :, :]                      # [128, ne, et] contig
                    if route == "A":
                        junk = junk_pool.tile([128, ne, et], fp16, tag="junk")
                        nc.vector.scalar_tensor_tensor(
                            junk[:], epj, 1.0, wv,
                            op0=mybir.AluOpType.mult,
                            op1=mybir.AluOpType.mult,
                            accum_out=slot,
                        )
                    else:
                        prod = prod_pool.tile([128, dt], fp16, tag="prod")
                        nc.vector.tensor_tensor(
                            prod[:].rearrange("p (e c) -> p e c", e=ne, c=et),
                            epj, wv, op=mybir.AluOpType.mult)
                        if route == "H":
                            fw = dt // FOLD
                            psf = psumf_pool.tile([128, fw], fp32, tag="psf")
                            for g in range(FOLD):
                                nc.tensor.matmul(
                                    psf[:],
                                    idmat[:],
                                    prod[:, g * fw:(g + 1) * fw],
                                    start=(g == 0),
                                    stop=(g == FOLD - 1),
                                )
                            nc.vector.tensor_reduce(
                                slot, psf[:], axis=mybir.AxisListType.X,
                                op=mybir.AluOpType.add,
                            )
                        else:  # C: ACT copy-accumulate reduce
                            junk2 = junk_pool.tile([128, dt], fp16, tag="junk2")
                            nc.scalar.activation(
                                junk2[:], prod[:],
                                mybir.ActivationFunctionType.Copy,
                                scale=1.0,
                                accum_out=slot,
                            )

            res = acc_pool.tile([128, NQ + 1], fp32, tag="res")
            for j in range(NQ):
                nc.vector.tensor_reduce(
                    res[:, j:j + 1],
                    master_aw[:, j * n_tiles:(j + 1) * n_tiles],
                    axis=mybir.AxisListType.X,
                    op=mybir.AluOpType.add,
                )
            nc.vector.tensor_reduce(
                res[:, NQ:NQ + 1], master_se[:], axis=mybir.AxisListType.X,
                op=mybir.AluOpType.add,
            )
            nc.sync.dma_start(out_dram[:], res[:])

    nc.compile()
    return nc


def _pack_core(e_core, dt, et):
    # [Nc, 64] -> A[r, dh, j, n] = E[8n+r, 16j+dh] -> [k, (t, e, j, c)]
    ncols = e_core.shape[0] // R
    n_tiles = ncols // dt
    ne = dt // et
    a = e_core.reshape(ncols, R, NQ, DQ).transpose(1, 3, 2, 0)     # [r, dh, j, n]
    a = a.reshape(128, NQ, n_tiles, ne, et).transpose(0, 2, 3, 1, 4)  # [k,t,e,j,c]
    return np.ascontiguousarray(a.reshape(128, NQ * ncols)).astype(np.float16)


def _make_consts(query):
    c_shift = float(6.0 * np.linalg.norm(query.astype(np.float64)))
    q16 = query.astype(np.float16)
    qmat = np.zeros((128, NQ * 128), dtype=np.float16)
    for j in range(NQ):
        for r in range(R):
            qmat[r * DQ:(r + 1) * DQ, j * 128 + r * DQ:j * 128 + (r + 1) * DQ] = \
                q16[j * DQ:(j + 1) * DQ][:, None]
    cshift = np.full((128, 1), -c_shift, dtype=np.float32)
    idmat = np.eye(128, dtype=np.float16)
    return qmat, cshift, idmat


def get_nc():
    key = (NCOLS, DT, ET, "".join(ROUTES))
    if key not in _compiled:
        _compiled[key] = _build_nc(NCOLS, DT, ET, ROUTES)
    return _compiled[key]


def make_in_maps(embeddings, query):
    embeddings = np.asarray(embeddings, dtype=np.float32)
    query = np.asarray(query, dtype=np.float32)
    qmat, cshift, idmat = _make_consts(query)
    in_maps = []
    for c in range(N_CORES):
        e_core = embeddings[c * N_PER_CORE:(c + 1) * N_PER_CORE]
        in_maps.append({
            "epack": _pack_core(e_core, DT, ET),
            "qmat": qmat,
            "cshift": cshift,
            "idmat": idmat,
        })
    return in_maps


def combine(results):
    wsum = np.zeros(D, dtype=np.float64)
    sumexp = 0.0
    for r_ in results:
        out = r_["out"].astype(np.float64)          # [128, 5]
        for j in range(NQ):
            wsum[j * DQ:(j + 1) * DQ] += out[:, j].reshape(R, DQ).sum(axis=0)
        sumexp += out[::DQ, NQ].sum()               # one rep per r-block
    return (wsum / sumexp).astype(np.float32)


def kernel(embeddings, query):
    from concourse.bass_utils import run_bass_kernel_spmd

    nc = get_nc()
    in_maps = make_in_maps(embeddings, query)

    res = None
    for attempt in range(3):
        try:
            res = run_bass_kernel_spmd(nc, in_maps, list(range(N_CORES)))
            break
        except Exception:
            if attempt == 2:
                raise
    return combine(res.results)


# revision 12
# speedup vs baseline: 1.0672x; 1.0432x over previous
"""Attention pooling kernel for Trainium2 (8 NeuronCores), v6.

Computes: scores = E @ q; w = softmax(scores); out = w @ E
for E [N=2097152, 64] fp32, q [64] fp32.

v10 = v8 + one-tile software-pipelined emission:
  - 8-row packing: partition k = (r, dh), r in [0,8), dh in [0,16);
    d split into 4 quarters j; scores accumulate 4 matmuls (qmat_j)
    into PSUM with start/stop flags.
  - DRAM layout [k, (t, e, j, c)]: DMA tile t (DT=4096 n-cols) is
    fetched as 4 strips of 1MB (one per exp-tile e of ET=1024), so the
    first matmuls start after ~1MB instead of 4MB, and 32 smaller
    transfers pack the 16 DMA engines more smoothly.
  - ep tiles are 4D [128, e, j, c]; the weighted-sum unit for quarter
    j reads the strided view ep[:, :, j, :] (innermost contiguous, so
    DVE 2x mode is preserved).
  - Weighted-sum routes (A=DVE stt fused, H=DVE 2x mult + PE 16:1
    identity-matmul PSUM fold + small DVE reduce, C=DVE 2x mult + ACT
    Copy accum_out reduce), mix A2/H18/C12, all fp16.
  - Host combine identical to v3.
"""

import sys

sys.path.insert(0, "/opt/trn_rl_repo")

import numpy as np

N_TOTAL = 2097152
D = 64
N_CORES = 8
N_PER_CORE = N_TOTAL // N_CORES          # 262144
R = 8                                    # rows packed per n-column
NQ = 4                                   # d quarters
DQ = D // NQ                             # 16 d's per quarter
NCOLS = N_PER_CORE // R                  # 32768 n-columns per core
MM_N = 512                               # scores matmul free dim
DT = 4096                                # n-columns per DMA tile
ET = 1024                                # n-columns per exp/PSUM tile
NE = DT // ET                            # 4 strips per tile
N_TILES = NCOLS // DT                    # 8
FOLD = 8                                 # H-route fold ratio

# route per unit (unit = t * NQ + j):
# A=DVE stt, H=DVE mult + PE fold + DVE reduce, C=DVE mult + ACT reduce
ROUTES = list("HCHAHCHC" * 4)
ROUTES[3] = "H"
ROUTES[19] = "H"                         # -> A2 H18 C12

_compiled = {}


def _build_nc(ncols, dt, et, routes):
    import concourse.bacc as bacc
    import concourse.bass as bass
    import concourse.mybir as mybir
    import concourse.tile as tile

    fp32 = mybir.dt.float32
    fp16 = mybir.dt.float16

    n_tiles = ncols // dt
    ne = dt // et
    blk = NQ * dt                        # elements per DMA tile per partition
    strip = NQ * et                      # elements per strip per partition

    nc = bacc.Bacc()
    ep_dram = nc.declare_dram_parameter("epack", [128, NQ * ncols], fp16, isOutput=False)
    qmat_dram = nc.declare_dram_parameter("qmat", [128, NQ * 128], fp16, isOutput=False)
    idmat_dram = nc.declare_dram_parameter("idmat", [128, 128], fp16, isOutput=False)
    cshift_dram = nc.declare_dram_parameter("cshift", [128, 1], fp32, isOutput=False)
    out_dram = nc.declare_dram_parameter("out", [128, NQ + 1], fp32, isOutput=True)

    with tile.TileContext(nc) as tc:
        with (
            tc.tile_pool(name="const", bufs=1) as const_pool,
            tc.tile_pool(name="ep", bufs=4) as ep_pool,
            tc.tile_pool(name="w", bufs=3) as w_pool,
            tc.tile_pool(name="junk", bufs=1) as junk_pool,
            tc.tile_pool(name="prod", bufs=3) as prod_pool,
            tc.tile_pool(name="acc", bufs=1) as acc_pool,
            tc.tile_pool(name="psum", bufs=2, space=bass.MemorySpace.PSUM) as psum_pool,
            tc.tile_pool(name="psumf", bufs=2, space=bass.MemorySpace.PSUM) as psumf_pool,
        ):
            qmat = const_pool.tile([128, NQ * 128], fp16, tag="qmat")
            idmat = const_pool.tile([128, 128], fp16, tag="idmat")
            cshift = const_pool.tile([128, 1], fp32, tag="cshift")
            nc.sync.dma_start(qmat[:], qmat_dram[:])
            nc.sync.dma_start(idmat[:], idmat_dram[:])
            nc.sync.dma_start(cshift[:], cshift_dram[:])

            n_acts = ncols // et
            master_se = acc_pool.tile([128, n_acts], fp32, tag="master_se")
            master_aw = acc_pool.tile([128, NQ * n_tiles], fp32, tag="master_aw")

            def emit_scores(t):
                ep = ep_pool.tile([128, ne, NQ, et], fp16, tag="ep")
                for e in range(ne):
                    nc.sync.dma_start(
                        ep[:, e, :, :].rearrange("p j c -> p (j c)"),
                        ep_dram[:, t * blk + e * strip:t * blk + (e + 1) * strip],
                    )
                w_sb = w_pool.tile([128, ne, et], fp16, tag="w")
                for e in range(ne):
                    ps = psum_pool.tile([128, et], fp32, tag="ps")
                    for j in range(NQ):
                        for c0 in range(0, et, MM_N):
                            nc.tensor.matmul(
                                ps[:, c0:c0 + MM_N],
                                qmat[:, j * 128:(j + 1) * 128],
                                ep[:, e, j, c0:c0 + MM_N],
                                start=(j == 0),
                                stop=(j == NQ - 1),
                            )
                    nc.scalar.activation(
                        w_sb[:, e, :],
                        ps[:],
                        mybir.ActivationFunctionType.Exp,
                        bias=cshift[:, 0:1],
                        scale=1.0,
                        accum_out=master_se[:, t * ne + e:t * ne + e + 1],
                    )
                return ep, w_sb

            def emit_units(t, ep, w_sb):
                # pass 1: all DVE products (big ops first, keeps DVE dense)
                prods = {}
                for j in range(NQ):
                    unit = t * NQ + j
                    route = routes[unit]
                    slot = master_aw[:, j * n_tiles + t:j * n_tiles + t + 1]
                    epj = ep[:, :, j, :]                    # [128, ne, et] strided
                    wv = w_sb[:, :, :]                      # [128, ne, et] contig
                    if route == "A":
                        junk = junk_pool.tile([128, ne, et], fp16, tag="junk")
                        nc.vector.scalar_tensor_tensor(
                            junk[:], epj, 1.0, wv,
                            op0=mybir.AluOpType.mult,
                            op1=mybir.AluOpType.mult,
                            accum_out=slot,
                        )
                    else:
                        prod = prod_pool.tile([128, dt], fp16, tag="prod")
                        nc.vector.tensor_tensor(
                            prod[:].rearrange("p (e c) -> p e c", e=ne, c=et),
                            epj, wv, op=mybir.AluOpType.mult)
                        prods[j] = (route, prod, slot)
                # pass 2: reduces (PE folds + DVE reduce / ACT copies)
                for j, (route, prod, slot) in prods.items():
                    if route == "H":
                        fw = dt // FOLD
                        psf = psumf_pool.tile([128, fw], fp32, tag="psf")
                        for g in range(FOLD):
                            nc.tensor.matmul(
                                psf[:],
                                idmat[:],
                                prod[:, g * fw:(g + 1) * fw],
                                start=(g == 0),
                                stop=(g == FOLD - 1),
                            )
                        nc.vector.tensor_reduce(
                            slot, psf[:], axis=mybir.AxisListType.X,
                            op=mybir.AluOpType.add,
                        )
                    else:  # C: ACT copy-accumulate reduce
                        junk2 = junk_pool.tile([128, dt], fp16, tag="junk2")
                        nc.scalar.activation(
                            junk2[:], prod[:],
                            mybir.ActivationFunctionType.Copy,
                            scale=1.0,
                            accum_out=slot,
                        )

            # software-pipelined emission: units of tile t-1 are emitted after
            # the scores of tile t, so the PE stream never has folds(t) between
            # scores(t) and scores(t+1) (in-order head-of-line blocking).
            prev = None
            for t in range(n_tiles):
                cur = emit_scores(t)
                if prev is not None:
                    emit_units(t - 1, *prev)
                prev = cur
            emit_units(n_tiles - 1, *prev)

            res = acc_pool.tile([128, NQ + 1], fp32, tag="res")
            for j in range(NQ):
                nc.vector.tensor_reduce(
                    res[:, j:j + 1],
                    master_aw[:, j * n_tiles:(j + 1) * n_tiles],
                    axis=mybir.AxisListType.X,
                    op=mybir.AluOpType.add,
                )
            nc.vector.tensor_reduce(
                res[:, NQ:NQ + 1], master_se[:], axis=mybir.AxisListType.X,
                op=mybir.AluOpType.add,
            )
            nc.sync.dma_start(out_dram[:], res[:])

    nc.compile()
    return nc


def _pack_core(e_core, dt, et):
    # [Nc, 64] -> A[r, dh, j, n] = E[8n+r, 16j+dh] -> [k, (t, e, j, c)]
    ncols = e_core.shape[0] // R
    n_tiles = ncols // dt
    ne = dt // et
    a = e_core.reshape(ncols, R, NQ, DQ).transpose(1, 3, 2, 0)     # [r, dh, j, n]
    a = a.reshape(128, NQ, n_tiles, ne, et).transpose(0, 2, 3, 1, 4)  # [k,t,e,j,c]
    return np.ascontiguousarray(a.reshape(128, NQ * ncols)).astype(np.float16)


def _make_consts(query):
    c_shift = float(6.0 * np.linalg.norm(query.astype(np.float64)))
    q16 = query.astype(np.float16)
    qmat = np.zeros((128, NQ * 128), dtype=np.float16)
    for j in range(NQ):
        for r in range(R):
            qmat[r * DQ:(r + 1) * DQ, j * 128 + r * DQ:j * 128 + (r + 1) * DQ] = \
                q16[j * DQ:(j + 1) * DQ][:, None]
    cshift = np.full((128, 1), -c_shift, dtype=np.float32)
    idmat = np.eye(128, dtype=np.float16)
    return qmat, cshift, idmat


def get_nc():
    key = (NCOLS, DT, ET, "".join(ROUTES))
    if key not in _compiled:
        _compiled[key] = _build_nc(NCOLS, DT, ET, ROUTES)
    return _compiled[key]


def make_in_maps(embeddings, query):
    embeddings = np.asarray(embeddings, dtype=np.float32)
    query = np.asarray(query, dtype=np.float32)
    qmat, cshift, idmat = _make_consts(query)
    in_maps = []
    for c in range(N_CORES):
        e_core = embeddings[c * N_PER_CORE:(c + 1) * N_PER_CORE]
        in_maps.append({
            "epack": _pack_core(e_core, DT, ET),
            "qmat": qmat,
            "cshift": cshift,
            "idmat": idmat,
        })
    return in_maps


def combine(results):
    wsum = np.zeros(D, dtype=np.float64)
    sumexp = 0.0
    for r_ in results:
        out = r_["out"].astype(np.float64)          # [128, 5]
        for j in range(NQ):
            wsum[j * DQ:(j + 1) * DQ] += out[:, j].reshape(R, DQ).sum(axis=0)
        sumexp += out[::DQ, NQ].sum()               # one rep per r-block
    return (wsum / sumexp).astype(np.float32)


def kernel(embeddings, query):
    from concourse.bass_utils import run_bass_kernel_spmd

    nc = get_nc()
    in_maps = make_in_maps(embeddings, query)

    res = None
    for attempt in range(3):
        try:
            res = run_bass_kernel_spmd(nc, in_maps, list(range(N_CORES)))
            break
        except Exception:
            if attempt == 2:
                raise
    return combine(res.results)
